# revision 1
# baseline (speedup 1.0000x reference)
"""Trainium2 Bass kernel for the ExpertVectorSystem MoE-routing problem.

Reference computation (all fp32):
    we = expert_weights @ expert_vectors              # [B, D]
    for each layer i (8 layers, rank r_i):
        h_i   = relu(we @ w1_i + b1_i)                # [B, 2r]
        out_i = tanh(h_i @ w2_i + b2_i) * 0.1         # [B, r]
    out = concat(out_i, axis=-1)                      # [B, sum(r)]

Strategy: data-parallel over the batch across 8 NeuronCores (2048 rows
each); the tiny expert_vectors / per-layer MLP weights are replicated.

All matmuls run in float32r (fp32 bits, reduced-precision PE mode): at
moving-dim >= 256 it streams 1 row/cycle like bf16 (4x faster than
strict fp32) while measuring ~16x more accurate than bf16.

Per-core device program (fp32r matmuls, fp32 PSUM accumulation),
measured 671 us HW exec / rel err 3.3e-4 vs the fp32 reference:
  phase 0: weT_ext [65, 2048] = (v_aug^T @ ewT_ext) on the PE from a
           host-pre-transposed ewT shard; the homogeneous ones-row folds
           b1 into an augmented w1 (K=64 -> 65).  ~4us of dummy warm-up
           matmuls run during the startup DMAs so the HAM clock gate
           reaches 8/8 (2.4 GHz) before the layers start.
  per (layer, batch-group of 512 columns) pair:
    stage 1: hT chunks [128, 512] = w1_aug_chunk^T @ weT_ext (K=65,
             N=512); relu drains alternate ScalarE/VectorE into SBUF.
             Emitted one pair ahead, inside the previous pair's stage-2,
             so the relu latency hides under PE work.
    stage 2: out tiles [128(batch), r-chunk in [256,512]] accumulated
             over the 2r/128 K-chunks in PSUM, looped c-outer/rc-inner
             so h/w2 chunk slots free progressively; then tanh on
             ScalarE, *0.1 on VectorE, DMA to the output column slice.

If any b2 is nonzero, the same homogeneous trick adds one extra K-chunk
whose first h row is constant 1 and whose w2 rows carry b2 (zero for
this problem's fixed setup, so normally off).
"""

import contextlib
import ctypes
import os
import sys
import types

import numpy as np

import concourse.bass as bass
import concourse.mybir as mybir
import concourse.tile as tile
from concourse.bass_utils import run_bass_kernel_spmd

B = 16384
E = 16
D = 64
RANKS = [256, 384, 512, 640, 768, 896, 1024, 1152]
STRENGTH = 0.1
NCORES = 8
BL = B // NCORES          # 2048 rows per core
GCOLS = 512               # batch columns per stage-1 group
NGROUPS = BL // GCOLS     # 4
NTILES_PER_GROUP = GCOLS // 128  # 4

F32R = mybir.dt.float32r
F32 = mybir.dt.float32

OUT_COLS = sum(RANKS)     # 5888


def _split_excess_waits(nc):
    """Rewrite instructions carrying >1 sync wait.

    The walrus build in this container accepts at most ONE sync wait per
    instruction ("Too many sync wait commands", CoreV*GenImpl
    setupSyncWait), while Tile's wait assignment freely attaches several.
    Hoist the extra waits onto standalone InstEventSemaphore instructions
    (what BassEngine.wait_ge emits) inserted immediately before the
    instruction on the same engine — same-engine program order makes this
    semantically identical.
    """
    n_split = 0
    for f in nc.m.functions:
        for bb in f.blocks:
            out = []
            dirty = False
            for ins in bb.instructions:
                si = ins.sync_info
                waits = list(si.on_wait) if si is not None else []
                if len(waits) > 1:
                    dirty = True
                    for k, w in enumerate(waits[:-1]):
                        out.append(
                            mybir.InstEventSemaphore(
                                name=f"{ins.name}_xw{k}",
                                engine=ins.engine,
                                ins=[],
                                outs=[],
                                sync_info=mybir.SyncInfo(
                                    on_wait=[w], on_update=[]
                                ),
                            )
                        )
                        n_split += 1
                    ins.sync_info = mybir.SyncInfo(
                        on_wait=[waits[-1]], on_update=list(si.on_update)
                    )
                out.append(ins)
            if dirty:
                bb.instructions = out
    return n_split


def _rchunks(r):
    """Split a layer's output width r into nearly-even chunks <= 512.

    Every chunk ends up in [256, 512] for the given ranks, which keeps
    float32r matmuls at the full 1-row/cycle rate.
    """
    n = -(-r // 512)
    sizes = []
    rem = r
    for i in range(n):
        s = -(-rem // (n - i))
        sizes.append(s)
        rem -= s
    offs = [0]
    for s in sizes[:-1]:
        offs.append(offs[-1] + s)
    return list(zip(offs, sizes))


def _build_program_biased(with_b2: bool):
    """Biased fallback: b1 folded via K=65 homogeneous row; optional b2
    via an extra K-chunk.  Used only when the inputs carry nonzero biases
    (never for this problem's fixed setup, where both are zeros)."""
    kcs = [2 * r // 128 + (1 if with_b2 else 0) for r in RANKS]
    w1_cols = [kc * 128 for kc in kcs]           # per-layer w1_aug col count
    W1TOT = sum(w1_cols)

    nc = bass.Bass()
    # ewT carries an appended ones-row; v_aug is block-diagonal so the
    # phase-0 matmul emits weT_ext = [[we^T], [ones]] directly (no memset:
    # this walrus rejects Memset on float32r).
    ewT_d = nc.declare_dram_parameter("ewT", [E + 1, BL], F32R, isOutput=False)
    v_d = nc.declare_dram_parameter("v", [E + 1, D + 1], F32R, isOutput=False)
    w1_d = nc.declare_dram_parameter("w1cat", [D + 1, W1TOT], F32R, isOutput=False)
    w2_d = [
        nc.declare_dram_parameter(f"w2_{i}", [128, kcs[i] * RANKS[i]], F32R,
                                  isOutput=False)
        for i in range(len(RANKS))
    ]
    out_d = nc.declare_dram_parameter("out", [BL, OUT_COLS], F32, isOutput=True)

    col_offs = [sum(RANKS[:i]) for i in range(len(RANKS))]

    with tile.TileContext(nc) as tc:
        with (
            tc.tile_pool(name="const", bufs=1) as cpool,
            tc.tile_pool(name="hpsum", bufs=4, space="PSUM") as hpsum,
            tc.tile_pool(name="opsum", bufs=2, space="PSUM") as opsum,
            tc.tile_pool(name="w1", bufs=2) as w1pool,
            tc.tile_pool(name="w2", bufs=1) as w2pool,
            tc.tile_pool(name="h", bufs=2) as hpool,
            tc.tile_pool(name="osb", bufs=6) as osb,
        ):
            # ---- phase 0: load constants, compute weT_ext [65, BL] ----
            v_sb = cpool.tile([E + 1, D + 1], F32R, name="v_sb")
            nc.sync.dma_start(v_sb[:], v_d[:])

            # PE warm-up: ~4us of dummy matmuls on v_sb (arrives ~instantly)
            # while the bulk input DMAs stream, so the HAM clock gate is at
            # 8/8 (2.4 GHz) when the real layers begin and the PE never
            # idles >3.4us at the start.
            for k in range(72):
                warm = hpsum.tile([64, 64], F32, tag="hp", name=f"warm_{k}")
                nc.tensor.matmul(
                    warm[:], v_sb[:, 0:64], v_sb[:, 0:64], start=True, stop=True
                )

            weT = cpool.tile([D + 1, BL], F32R, name="weT")
            ewT_sb = cpool.tile([E + 1, BL], F32R, name="ewT_sb")
            nc.sync.dma_start(ewT_sb[:], ewT_d[:])

            def load_w1(li):
                off = sum(w1_cols[:li])
                t = w1pool.tile([D + 1, w1_cols[li]], F32R, tag="w1",
                                name=f"w1_{li}")
                nc.sync.dma_start(t[:], w1_d[:, off:off + w1_cols[li]])
                return t

            def load_w2(li):
                r = RANKS[li]
                tiles = []
                for c in range(kcs[li]):
                    # chunks 0-1 double-buffered: their DMAs start before
                    # the previous layer's last reads finish, covering the
                    # first stage-2 matmuls right after a layer boundary
                    t = w2pool.tile([128, r], F32R, tag=f"w2_{c}",
                                    bufs=(2 if c < 1 else 1),
                                    name=f"w2_{li}_{c}")
                    nc.sync.dma_start(t[:], w2_d[li][:, c * r:(c + 1) * r])
                    tiles.append(t)
                return tiles

            # critical-path order: layer-0 weights right after ewT
            w1_sb = {0: load_w1(0)}
            w2_sb = {0: load_w2(0)}

            for g in range(NGROUPS):
                wp = hpsum.tile([D + 1, GCOLS], F32, tag="hp", name="wp")
                nc.tensor.matmul(
                    wp[:], v_sb[:], ewT_sb[:, g * GCOLS:(g + 1) * GCOLS],
                    start=True, stop=True,
                )
                nc.vector.tensor_copy(
                    weT[0:D + 1, g * GCOLS:(g + 1) * GCOLS], wp[:]
                )

            # ---- main sweep over (layer, batch-group) pairs ----
            # stage-1 of pair k+1 is emitted one chunk at a time, spread
            # through pair k's j=1..3 sweeps, so its relu (alternating
            # ACT/DVE at ~670ns/tile each) always keeps pace with the PE
            # and the 4 hp PSUM slots never back up.
            def stage1_units(li, g, h_sb):
                for c in range(kcs[li]):
                    def unit(c=c):
                        hp = hpsum.tile([128, GCOLS], F32, tag="hp",
                                        name=f"hp_{li}_{g}_{c}")
                        nc.tensor.matmul(
                            hp[:],
                            w1_sb[li][:, c * 128:(c + 1) * 128],
                            weT[:, g * GCOLS:(g + 1) * GCOLS],
                            start=True, stop=True,
                        )
                        ht = hpool.tile([128, GCOLS], F32R, tag=f"h_{c}",
                                        name=f"h_{li}_{g}_{c}")
                        if c % 2 == 0:
                            nc.scalar.activation(
                                ht[:], hp[:], mybir.ActivationFunctionType.Relu
                            )
                        else:
                            nc.vector.tensor_scalar_max(ht[:], hp[:], 0.0)
                        h_sb.append(ht)
                    yield unit

            pairs = [(li, g) for li in range(len(RANKS)) for g in range(NGROUPS)]
            h_cur = []
            for u in stage1_units(0, 0, h_cur):
                u()
            for idx, (li, g) in enumerate(pairs):
                r = RANKS[li]
                kc = kcs[li]
                rch = _rchunks(r)
                col_off = col_offs[li]
                nxt = pairs[idx + 1] if idx + 1 < len(pairs) else None
                h_nxt = []
                units = iter(())
                n_units = 0
                if nxt is not None:
                    nli, ng = nxt
                    if nli != li:
                        w1_sb[nli] = load_w1(nli)
                        w2_sb[nli] = load_w2(nli)
                    units = stage1_units(nli, ng, h_nxt)
                    n_units = kcs[nli]
                for j in range(NTILES_PER_GROUP):
                    row0 = g * GCOLS + j * 128
                    # c-outer / rc-inner: each h chunk (and w2 chunk) sees
                    # its last read early in the j=3 sweep, freeing slots
                    # progressively for the next pair / next layer's DMAs.
                    ops = [
                        opsum.tile([128, rc_sz], F32, tag=f"op{ri % 2}",
                                   name=f"op_{li}_{g}_{j}_{ri}")
                        for ri, (rc_off, rc_sz) in enumerate(rch)
                    ]
                    for c in range(kc):
                        for ri, (rc_off, rc_sz) in enumerate(rch):
                            nc.tensor.matmul(
                                ops[ri][:],
                                h_cur[c][:, j * 128:(j + 1) * 128],
                                w2_sb[li][c][:, rc_off:rc_off + rc_sz],
                                start=(c == 0), stop=(c == kc - 1),
                            )
                    for ri, (rc_off, rc_sz) in enumerate(rch):
                        ot = osb.tile([128, rc_sz], F32, tag="ot",
                                      name=f"ot_{li}_{g}_{j}_{ri}")
                        nc.scalar.activation(
                            ot[:], ops[ri][:], mybir.ActivationFunctionType.Tanh
                        )
                        nc.vector.tensor_scalar_mul(ot[:], ot[:], STRENGTH)
                        nc.sync.dma_start(
                            out_d[row0:row0 + 128,
                                  col_off + rc_off:col_off + rc_off + rc_sz],
                            ot[:],
                        )
                    # stage-1 lump for the next pair goes AFTER j=1's tanh
                    # block: ACT then drains the op PSUM banks before the
                    # relu burst, so j>=2 sweeps never wait on bank reuse.
                    if j == 1:
                        for u in units:
                            u()
                for u in units:
                    u()
                h_cur = h_nxt
    _split_excess_waits(nc)
    return nc


def _build_program_packed():
    """No-bias fast path: K=64 stage-1 matmuls packed two-at-a-time into
    disjoint PE row-group halves via tile_position, and the next pair's
    stage-1 spread through the current stage-2 sweep so the relu drain
    (split ACT/DVE) stays off the PE's critical path."""
    kcs = [2 * r // 128 for r in RANKS]
    w1_cols = [kc * 64 for kc in kcs]            # packed: 2 chunks per 128 cols
    W1TOT = sum(w1_cols)

    nc = bass.Bass()
    ewT_d = nc.declare_dram_parameter("ewT", [E, BL], F32R, isOutput=False)
    v_d = nc.declare_dram_parameter("v", [E, D], F32R, isOutput=False)
    # w1cat packed: [128, sum(kc/2 * 128)]: rows 0:64 = even chunk,
    # rows 64:128 = odd chunk of each 128-col block
    w1_d = nc.declare_dram_parameter("w1cat", [128, W1TOT], F32R, isOutput=False)
    w2_d = [
        nc.declare_dram_parameter(f"w2_{i}", [128, kcs[i] * RANKS[i]], F32R,
                                  isOutput=False)
        for i in range(len(RANKS))
    ]
    out_d = nc.declare_dram_parameter("out", [BL, OUT_COLS], F32, isOutput=True)

    col_offs = [sum(RANKS[:i]) for i in range(len(RANKS))]

    with tile.TileContext(nc) as tc:
        with (
            tc.tile_pool(name="const", bufs=1) as cpool,
            tc.tile_pool(name="hpsum", bufs=4, space="PSUM") as hpsum,
            tc.tile_pool(name="opsum", bufs=2, space="PSUM") as opsum,
            tc.tile_pool(name="w1", bufs=2) as w1pool,
            tc.tile_pool(name="w2", bufs=1) as w2pool,
            tc.tile_pool(name="h", bufs=2) as hpool,
            tc.tile_pool(name="osb", bufs=6) as osb,
        ):
            v_sb = cpool.tile([E, D], F32R, name="v_sb")
            nc.sync.dma_start(v_sb[:], v_d[:])

            # PE warm-up while bulk DMAs stream (HAM at 8/8 for the layers)
            for k in range(72):
                warm = hpsum.tile([64, 64], F32, tag="hp", name=f"warm_{k}")
                nc.tensor.matmul(
                    warm[:], v_sb[:, 0:64], v_sb[:, 0:64], start=True, stop=True
                )

            # weT duplicated into both partition halves so packed stage-1
            # matmuls can stream it into either PE row-group half.
            weT2 = cpool.tile([128, BL], F32R, name="weT2")
            ewT_sb = cpool.tile([E, BL], F32R, name="ewT_sb")
            nc.sync.dma_start(ewT_sb[:], ewT_d[:])

            def load_w1(li):
                off = sum(w1_cols[:li])
                t = w1pool.tile([128, w1_cols[li]], F32R, tag="w1",
                                name=f"w1_{li}")
                nc.sync.dma_start(t[:], w1_d[:, off:off + w1_cols[li]])
                return t

            def load_w2(li):
                r = RANKS[li]
                tiles = []
                for c in range(kcs[li]):
                    t = w2pool.tile([128, r], F32R, tag=f"w2_{c}",
                                    name=f"w2_{li}_{c}")
                    nc.sync.dma_start(t[:], w2_d[li][:, c * r:(c + 1) * r])
                    tiles.append(t)
                return tiles

            w1_sb = {0: load_w1(0)}
            w2_sb = {0: load_w2(0)}

            for g in range(NGROUPS):
                wp = hpsum.tile([D, GCOLS], F32, tag="hp", name="wp")
                nc.tensor.matmul(
                    wp[:], v_sb[:], ewT_sb[:, g * GCOLS:(g + 1) * GCOLS],
                    start=True, stop=True,
                )
                nc.vector.tensor_copy(
                    weT2[0:D, g * GCOLS:(g + 1) * GCOLS], wp[:]
                )
                nc.scalar.copy(
                    weT2[D:2 * D, g * GCOLS:(g + 1) * GCOLS], wp[:]
                )

            def stage1_units(li, g, h_sb):
                """Yield thunks; each emits one packed pair of stage-1
                matmuls (PE row-groups 0-1 / 2-3 run them concurrently)
                plus their relu drains on ACT and DVE."""
                for cp in range(kcs[li] // 2):
                    def unit(cp=cp):
                        hp_e = hpsum.tile([128, GCOLS], F32, tag="hp",
                                          name=f"hpe_{li}_{g}_{cp}")
                        nc.tensor.matmul(
                            hp_e[:],
                            w1_sb[li][0:64, cp * 128:(cp + 1) * 128],
                            weT2[0:64, g * GCOLS:(g + 1) * GCOLS],
                            start=True, stop=True,
                        )
                        hp_o = hpsum.tile([128, GCOLS], F32, tag="hp",
                                          name=f"hpo_{li}_{g}_{cp}")
                        nc.tensor.matmul(
                            hp_o[:],
                            w1_sb[li][64:128, cp * 128:(cp + 1) * 128],
                            weT2[64:128, g * GCOLS:(g + 1) * GCOLS],
                            start=True, stop=True,
                        )
                        ht_e = hpool.tile([128, GCOLS], F32R, tag=f"h_{2*cp}",
                                          name=f"h_{li}_{g}_{2*cp}")
                        nc.scalar.activation(
                            ht_e[:], hp_e[:], mybir.ActivationFunctionType.Relu
                        )
                        ht_o = hpool.tile([128, GCOLS], F32R, tag=f"h_{2*cp+1}",
                                          name=f"h_{li}_{g}_{2*cp+1}")
                        nc.vector.tensor_scalar_max(ht_o[:], hp_o[:], 0.0)
                        h_sb.append(ht_e)
                        h_sb.append(ht_o)
                    yield unit

            pairs = [(li, g) for li in range(len(RANKS)) for g in range(NGROUPS)]
            h_cur = []
            for u in stage1_units(0, 0, h_cur):
                u()
            for idx, (li, g) in enumerate(pairs):
                r = RANKS[li]
                kc = kcs[li]
                rch = _rchunks(r)
                col_off = col_offs[li]
                nxt = pairs[idx + 1] if idx + 1 < len(pairs) else None
                h_nxt = []
                units = iter(())
                n_units = 0
                if nxt is not None:
                    nli, ng = nxt
                    if nli != li:
                        w1_sb[nli] = load_w1(nli)
                        w2_sb[nli] = load_w2(nli)
                    units = stage1_units(nli, ng, h_nxt)
                    n_units = kcs[nli] // 2
                for j in range(NTILES_PER_GROUP):
                    row0 = g * GCOLS + j * 128
                    ops = [
                        opsum.tile([128, rc_sz], F32, tag=f"op{ri % 2}",
                                   name=f"op_{li}_{g}_{j}_{ri}")
                        for ri, (rc_off, rc_sz) in enumerate(rch)
                    ]
                    for c in range(kc):
                        for ri, (rc_off, rc_sz) in enumerate(rch):
                            nc.tensor.matmul(
                                ops[ri][:],
                                h_cur[c][:, j * 128:(j + 1) * 128],
                                w2_sb[li][c][:, rc_off:rc_off + rc_sz],
                                start=(c == 0), stop=(c == kc - 1),
                            )
                    if j == 1:
                        for u in units:
                            u()
                    for ri, (rc_off, rc_sz) in enumerate(rch):
                        ot = osb.tile([128, rc_sz], F32, tag="ot",
                                      name=f"ot_{li}_{g}_{j}_{ri}")
                        nc.scalar.activation(
                            ot[:], ops[ri][:], mybir.ActivationFunctionType.Tanh
                        )
                        nc.vector.tensor_scalar_mul(ot[:], ot[:], STRENGTH)
                        nc.sync.dma_start(
                            out_d[row0:row0 + 128,
                                  col_off + rc_off:col_off + rc_off + rc_sz],
                            ot[:],
                        )
                for u in units:
                    u()
                h_cur = h_nxt
    _split_excess_waits(nc)
    return nc


_CACHE = {}


def _get_program(key):
    if key not in _CACHE:
        if key == "packed":
            _CACHE[key] = _build_program_packed()
        else:
            _CACHE[key] = _build_program_biased(key[1])
    return _CACHE[key]


def _prepare_inputs_packed(inputs):
    """Host-side prep for the no-bias packed program (all fp32 bits)."""
    ew = np.asarray(inputs["expert_weights"], dtype=np.float32)
    v = np.asarray(inputs["expert_vectors"], dtype=np.float32)
    ewT = np.ascontiguousarray(ew.T)                       # [E, B]

    w1_parts = []
    w2_parts = []
    for i, r in enumerate(RANKS):
        w1 = np.asarray(inputs[f"w1_{i}"], dtype=np.float32)   # [D, 2r]
        w2 = np.asarray(inputs[f"w2_{i}"], dtype=np.float32)   # [2r, r]
        kc = 2 * r // 128
        # [128, kc/2 * 128]: even chunk on partitions 0:64, odd on 64:128
        w1p = w1.reshape(D, kc // 2, 2, 128).transpose(2, 0, 1, 3)
        w1p = np.ascontiguousarray(w1p.reshape(2 * D, (kc // 2) * 128))
        w1_parts.append(w1p)
        w2_k = np.ascontiguousarray(
            w2.reshape(kc, 128, r).transpose(1, 0, 2).reshape(128, kc * r)
        )
        w2_parts.append(w2_k)
    w1cat = np.ascontiguousarray(np.concatenate(w1_parts, axis=1))

    in_maps = []
    for core in range(NCORES):
        m = {
            "ewT": np.ascontiguousarray(ewT[:, core * BL:(core + 1) * BL]),
            "v": v,
            "w1cat": w1cat,
        }
        for i in range(len(RANKS)):
            m[f"w2_{i}"] = w2_parts[i]
        in_maps.append(m)
    return in_maps


def _prepare_inputs(inputs, with_b2):
    """Host-side: transpose/augment and shard per core (all fp32 bits)."""
    ew = np.asarray(inputs["expert_weights"], dtype=np.float32)
    v = np.asarray(inputs["expert_vectors"], dtype=np.float32)

    # [E+1, B]: last row is all-ones (drives weT_ext's homogeneous row)
    ewT = np.concatenate([ew.T, np.ones((1, B), np.float32)], axis=0)
    # [E+1, D+1] block-diagonal: top-left = v, bottom-right = 1
    v_aug = np.zeros((E + 1, D + 1), np.float32)
    v_aug[:E, :D] = v
    v_aug[E, D] = 1.0

    w1_parts = []
    w2_parts = []
    for i, r in enumerate(RANKS):
        w1 = np.asarray(inputs[f"w1_{i}"], dtype=np.float32)   # [D, 2r]
        b1 = np.asarray(inputs[f"b1_{i}"], dtype=np.float32)   # [2r]
        w2 = np.asarray(inputs[f"w2_{i}"], dtype=np.float32)   # [2r, r]
        b2 = np.asarray(inputs[f"b2_{i}"], dtype=np.float32)   # [r]

        w1_aug = np.concatenate([w1, b1[None, :]], axis=0)     # [D+1, 2r]
        if with_b2:
            # extra 128 h-columns: first is the constant-1 unit
            # (weight col 0, b1 entry 1), rest identically zero.
            pad = np.zeros((D + 1, 128), np.float32)
            pad[D, 0] = 1.0
            w1_aug = np.concatenate([w1_aug, pad], axis=1)     # [D+1, 2r+128]
            w2pad = np.zeros((128, r), np.float32)
            w2pad[0, :] = b2
            w2 = np.concatenate([w2, w2pad], axis=0)           # [2r+128, r]
        kc = w2.shape[0] // 128
        w2_k = np.ascontiguousarray(
            w2.reshape(kc, 128, r).transpose(1, 0, 2).reshape(128, kc * r)
        )
        w1_parts.append(w1_aug)
        w2_parts.append(w2_k)
    w1cat = np.ascontiguousarray(np.concatenate(w1_parts, axis=1))

    in_maps = []
    for core in range(NCORES):
        m = {
            "ewT": np.ascontiguousarray(ewT[:, core * BL:(core + 1) * BL]),
            "v": v_aug,
            "w1cat": w1cat,
        }
        for i in range(len(RANKS)):
            m[f"w2_{i}"] = w2_parts[i]
        in_maps.append(m)
    return in_maps


def _install_ntff_hook():
    """Provide antenv.axon_hooks if the image lacks it (trace support).

    run_bass_kernel_spmd's axon trace path imports
    antenv.axon_hooks.get_axon_ntff_profile_hook; this container's antenv
    has no such module, so recreate the ctypes-based hook against the
    injected libaxon_pjrt.so (same as trn_agent_boot._ntff_profile_via_ctypes).
    """
    try:
        from antenv.axon_hooks import get_axon_ntff_profile_hook  # noqa: F401
        return
    except ImportError:
        pass
    so_path = "/opt/axon/libaxon_pjrt.so"
    hook = None
    if os.path.exists(so_path):
        lib = ctypes.CDLL(so_path)
        if hasattr(lib, "axon_start_nrt_profile"):
            lib.axon_start_nrt_profile.argtypes = [
                ctypes.POINTER(ctypes.c_int64),
                ctypes.c_size_t,
            ]
            lib.axon_start_nrt_profile.restype = ctypes.c_int64
            lib.axon_stop_nrt_profile.argtypes = [ctypes.c_char_p]
            lib.axon_stop_nrt_profile.restype = ctypes.c_int64

            @contextlib.contextmanager
            def _hook(output_dir, device_ids):
                import jax

                jax.devices()
                if device_ids:
                    ids = (ctypes.c_int64 * len(device_ids))(*device_ids)
                    rc = lib.axon_start_nrt_profile(ids, len(device_ids))
                else:
                    rc = lib.axon_start_nrt_profile(None, 0)
                if rc != 0:
                    raise RuntimeError(f"axon_start_nrt_profile rc={rc}")
                try:
                    yield
                finally:
                    n = lib.axon_stop_nrt_profile(str(output_dir).encode())
                    if n < 0:
                        raise RuntimeError(f"axon_stop_nrt_profile rc={n}")

            hook = _hook

    import antenv

    mod = types.ModuleType("antenv.axon_hooks")
    state = {"hook": hook}
    mod.get_axon_ntff_profile_hook = lambda: state["hook"]
    mod.set_axon_ntff_profile_hook = lambda h: state.__setitem__("hook", h)
    sys.modules["antenv.axon_hooks"] = mod
    antenv.axon_hooks = mod


def run(inputs, trace=False, tmpdir=None):
    """Run the kernel on all 8 cores; returns (full_output, BassKernelResults)."""
    with_b1 = any(
        np.any(np.asarray(inputs[f"b1_{i}"])) for i in range(len(RANKS))
    )
    with_b2 = any(
        np.any(np.asarray(inputs[f"b2_{i}"])) for i in range(len(RANKS))
    )
    if trace:
        _install_ntff_hook()
    # the packed no-bias variant measured slower on HW (row-group packed
    # stage-1 pairs mostly failed to overlap); the K=65 biased program is
    # both general and fastest, so use it unconditionally.
    nc = _get_program(("biased", with_b2))
    in_maps = _prepare_inputs(inputs, with_b2)
    del with_b1
    res = run_bass_kernel_spmd(
        nc, in_maps, core_ids=list(range(NCORES)), trace=trace, tmpdir=tmpdir
    )
    out = np.concatenate(
        [res.results[i]["out"] for i in range(NCORES)], axis=0
    ).astype(np.float32)
    return out, res


def kernel(**inputs) -> np.ndarray:
    out, _ = run(inputs, trace=False)
    return out



# revision 3
# speedup vs baseline: 1.1569x; 1.1569x over previous
"""Trainium2 Bass kernel for the ExpertVectorSystem MoE-routing problem.

Reference computation (all fp32):
    we = expert_weights @ expert_vectors              # [B, D]
    for each layer i (8 layers, rank r_i):
        h_i   = relu(we @ w1_i + b1_i)                # [B, 2r]
        out_i = tanh(h_i @ w2_i + b2_i) * 0.1         # [B, r]
    out = concat(out_i, axis=-1)                      # [B, sum(r)]

Strategy: data-parallel over the batch across 8 NeuronCores (2048 rows
each); the tiny per-layer MLP weights are replicated.

Key algebra: we = ew @ v has rank <= 16, so h = relu(ew_aug @ vw1_aug)
with vw1_aug = [[v @ w1], [b1]] ([17, 2r], host-folded).  Stage-1
contraction is K=17 instead of 65, so four chunks pack into the PE's
four 32-row tile groups (tile_position row tiling) and stream the same
moving ew columns concurrently: ~4x fewer stage-1 PE cycles.

All matmuls run in bf16 (fp32 PSUM accumulation): same 1-col/cycle PE
rate as fp32r but half the DMA/SBUF traffic, and bf16 stationaries get
Fast Weight Load so LDWEIGHTS hides completely under the matmul stream.
Simulated end-to-end rel err vs the fp32 reference: 4.3e-3 (fp8 would
be 4.9e-2 - fails the 2e-2 gate, so bf16 is the fastest legal dtype).

Stage-2 is computed transposed: out_pre.T[r, batch] accumulated as
(w2 chunk [128, 128-row-block]) stationary x (hT chunk [128, 512])
moving, so every matmul streams N=512 and every LDWEIGHTS (~96ns with
FWL) hides under the 213ns stream.  b2 rides the tanh activation's
per-partition bias port (free); the *0.1 scale and the final
[r, batch] -> [batch, r] transpose happen on the host.

Per-core schedule: per (layer, 512-col batch group) pair, stage-2 runs
r/128 PSUM accumulation groups (kc matmuls each); the next pair's
stage-1 quads are interleaved one-per-accumulation-group so the relu
drains (alternating ScalarE/VectorE) keep pace and PSUM never backs up.
"""

import contextlib
import ctypes
import os
import sys
import types

import numpy as np
import ml_dtypes

import concourse.bass as bass
import concourse.mybir as mybir
import concourse.tile as tile
from concourse.bass_utils import run_bass_kernel_spmd

B = 16384
E = 16
D = 64
RANKS = [256, 384, 512, 640, 768, 896, 1024, 1152]
STRENGTH = 0.1
NCORES = 8
BL = B // NCORES          # 2048 rows per core
GCOLS = 512               # batch columns per group
NGROUPS = BL // GCOLS     # 4

KC = [2 * r // 128 for r in RANKS]        # stage-2 K chunks per layer
NRB = [r // 128 for r in RANKS]           # output 128-row blocks per layer
QC = [(k + 3) // 4 for k in KC]           # stage-1 quads per layer
QOFF = [sum(QC[:i]) for i in range(len(RANKS))]
RBOFF = [sum(NRB[:i]) for i in range(len(RANKS))]
COLOFF = [sum(RANKS[:i]) for i in range(len(RANKS))]
NQ = sum(QC)              # 24 quad columns in vw1q
NRB_TOT = sum(NRB)        # 46

BF16 = mybir.dt.bfloat16
F32 = mybir.dt.float32
NP_BF16 = ml_dtypes.bfloat16

OUT_COLS = sum(RANKS)     # 5888


def _split_excess_waits(nc):
    """Rewrite instructions carrying >1 sync wait.

    The walrus build in this container accepts at most ONE sync wait per
    instruction ("Too many sync wait commands", CoreV*GenImpl
    setupSyncWait), while Tile's wait assignment freely attaches several.
    Hoist the extra waits onto standalone InstEventSemaphore instructions
    (what BassEngine.wait_ge emits) inserted immediately before the
    instruction on the same engine — same-engine program order makes this
    semantically identical.
    """
    n_split = 0
    for f in nc.m.functions:
        for bb in f.blocks:
            out = []
            dirty = False
            for ins in bb.instructions:
                si = ins.sync_info
                waits = list(si.on_wait) if si is not None else []
                if len(waits) > 1:
                    dirty = True
                    for k, w in enumerate(waits[:-1]):
                        out.append(
                            mybir.InstEventSemaphore(
                                name=f"{ins.name}_xw{k}",
                                engine=ins.engine,
                                ins=[],
                                outs=[],
                                sync_info=mybir.SyncInfo(
                                    on_wait=[w], on_update=[]
                                ),
                            )
                        )
                        n_split += 1
                    ins.sync_info = mybir.SyncInfo(
                        on_wait=[waits[-1]], on_update=list(si.on_update)
                    )
                out.append(ins)
            if dirty:
                bb.instructions = out
    return n_split


def _build_program():
    nc = bass.Bass()
    vw1q_d = nc.declare_dram_parameter("vw1q", [128, NQ * 128], BF16,
                                       isOutput=False)
    ewr_d = nc.declare_dram_parameter("ewr", [128, BL], BF16, isOutput=False)
    b2_d = nc.declare_dram_parameter("b2blk", [128, NRB_TOT], F32,
                                     isOutput=False)
    w2_d = [
        nc.declare_dram_parameter(f"w2_{i}", [128, KC[i] * RANKS[i]], BF16,
                                  isOutput=False)
        for i in range(len(RANKS))
    ]
    outT_d = nc.declare_dram_parameter("outT", [OUT_COLS, BL], BF16,
                                       isOutput=True)

    with tile.TileContext(nc) as tc:
        with (
            tc.tile_pool(name="const", bufs=1) as cpool,
            tc.tile_pool(name="hpsum", bufs=4, space="PSUM") as hpsum,
            tc.tile_pool(name="opsum", bufs=3, space="PSUM") as opsum,
            tc.tile_pool(name="w2", bufs=1) as w2pool,
            tc.tile_pool(name="h", bufs=2) as hpool,
            tc.tile_pool(name="osb", bufs=6) as osb,
        ):
            # ---- startup: DMAs + PE warm-up ----
            vw1q_sb = cpool.tile([128, NQ * 128], BF16, name="vw1q_sb")
            # first quad column lands first so warm-up can start ~immediately
            nc.sync.dma_start(vw1q_sb[:, 0:128], vw1q_d[:, 0:128])

            # ~4us of tiny matmuls in the same (32,128) tile mode as the
            # stage-1 quads, so the HAM clock gate reaches 8/8 (2.4 GHz)
            # during the startup DMAs and no mode-switch drain precedes
            # the first real quad.
            for k in range(64):
                warm = hpsum.tile([128, 64], F32, tag="hp", name=f"warm_{k}")
                nc.tensor.matmul(
                    warm[:], vw1q_sb[0:17, 0:128], vw1q_sb[0:17, 0:64],
                    start=True, stop=True, tile_position=(0, 0),
                )

            nc.sync.dma_start(vw1q_sb[:, 128:NQ * 128],
                              vw1q_d[:, 128:NQ * 128])
            ewr_sb = cpool.tile([128, BL], BF16, name="ewr_sb")
            nc.sync.dma_start(ewr_sb[:], ewr_d[:])
            b2_sb = cpool.tile([128, NRB_TOT], F32, name="b2_sb")
            nc.sync.dma_start(b2_sb[:], b2_d[:])

            def load_w2(li):
                r = RANKS[li]
                tiles = []
                for c in range(KC[li]):
                    t = w2pool.tile([128, r], BF16, tag=f"w2_{c}",
                                    bufs=(2 if c < 1 else 1),
                                    name=f"w2_{li}_{c}")
                    nc.sync.dma_start(t[:], w2_d[li][:, c * r:(c + 1) * r])
                    tiles.append(t)
                return tiles

            w2_sb = {0: load_w2(0)}

            # ---- stage 1: h chunks via 4-packed 32-row-tile matmuls ----
            def stage1_quads(li, g, h_sb):
                """Yield thunks; each emits one quad of K=17 matmuls into
                the PE's four 32-row tile groups (concurrent on HW) plus
                their relu drains split across ScalarE/VectorE."""
                qo = QOFF[li]
                for q in range(QC[li]):
                    def unit(q=q):
                        nt = min(4, KC[li] - 4 * q)
                        hps = []
                        for t in range(nt):
                            hp = hpsum.tile([128, GCOLS], F32, tag="hp",
                                            name=f"hp_{li}_{g}_{4*q+t}")
                            nc.tensor.matmul(
                                hp[:],
                                vw1q_sb[32 * t:32 * t + 17,
                                        (qo + q) * 128:(qo + q + 1) * 128],
                                ewr_sb[32 * t:32 * t + 17,
                                       g * GCOLS:(g + 1) * GCOLS],
                                start=True, stop=True,
                                tile_position=(32 * t, 0),
                            )
                            hps.append(hp)
                        for t, hp in enumerate(hps):
                            c = 4 * q + t
                            ht = hpool.tile([128, GCOLS], BF16, tag=f"h_{c}",
                                            name=f"h_{li}_{g}_{c}")
                            if c % 2 == 0:
                                nc.scalar.activation(
                                    ht[:], hp[:],
                                    mybir.ActivationFunctionType.Relu,
                                )
                            else:
                                nc.vector.tensor_scalar_max(ht[:], hp[:], 0.0)
                            h_sb.append(ht)
                    yield unit

            # ---- main sweep over (layer, batch-group) pairs ----
            pairs = [(li, g) for li in range(len(RANKS))
                     for g in range(NGROUPS)]
            h_cur = []
            for u in stage1_quads(0, 0, h_cur):
                u()
            for idx, (li, g) in enumerate(pairs):
                r = RANKS[li]
                kc = KC[li]
                nrb = NRB[li]
                nxt = pairs[idx + 1] if idx + 1 < len(pairs) else None
                h_nxt = []
                units = []
                if nxt is not None:
                    nli, ng = nxt
                    if nli != li:
                        w2_sb[nli] = load_w2(nli)
                    units = list(stage1_quads(nli, ng, h_nxt))
                ui = 0
                for rb in range(nrb):
                    op = opsum.tile([128, GCOLS], F32, tag="op",
                                    name=f"op_{li}_{g}_{rb}")
                    for c in range(kc):
                        nc.tensor.matmul(
                            op[:],
                            w2_sb[li][c][:, rb * 128:(rb + 1) * 128],
                            h_cur[c][:],
                            start=(c == 0), stop=(c == kc - 1),
                        )
                    # next pair's stage-1 quad between accumulation groups:
                    # its 4 relu drains get a full group (>= kc*213ns) to
                    # clear the 4 hp banks before the next quad needs them.
                    if ui < len(units):
                        units[ui]()
                        ui += 1
                    ot = osb.tile([128, GCOLS], BF16, tag="ot",
                                  name=f"ot_{li}_{g}_{rb}")
                    rbg = RBOFF[li] + rb
                    nc.scalar.activation(
                        ot[:], op[:], mybir.ActivationFunctionType.Tanh,
                        bias=b2_sb[:, rbg:rbg + 1],
                    )
                    row0 = COLOFF[li] + rb * 128
                    nc.sync.dma_start(
                        outT_d[row0:row0 + 128, g * GCOLS:(g + 1) * GCOLS],
                        ot[:],
                    )
                for u in units[ui:]:
                    u()
                h_cur = h_nxt
    _split_excess_waits(nc)
    return nc


_CACHE = {}


def _get_program():
    if "p" not in _CACHE:
        _CACHE["p"] = _build_program()
    return _CACHE["p"]


def _prepare_inputs(inputs):
    """Host-side marshalling: fold v@w1+b1 into the quad-packed stage-1
    stationary, chunk w2, build the replicated [ew^T; ones] bands."""
    ew = np.asarray(inputs["expert_weights"], dtype=np.float32)
    v = np.asarray(inputs["expert_vectors"], dtype=np.float32)

    vw1q = np.zeros((128, NQ * 128), np.float32)
    b2blk = np.zeros((128, NRB_TOT), np.float32)
    w2cat = []
    for i, r in enumerate(RANKS):
        w1 = np.asarray(inputs[f"w1_{i}"], dtype=np.float32)   # [D, 2r]
        b1 = np.asarray(inputs[f"b1_{i}"], dtype=np.float32)   # [2r]
        w2 = np.asarray(inputs[f"w2_{i}"], dtype=np.float32)   # [2r, r]
        b2 = np.asarray(inputs[f"b2_{i}"], dtype=np.float32)   # [r]
        vw1a = np.concatenate([v @ w1, b1[None, :]], axis=0)   # [17, 2r]
        for c in range(KC[i]):
            q, t = divmod(c, 4)
            vw1q[32 * t:32 * t + 17,
                 (QOFF[i] + q) * 128:(QOFF[i] + q + 1) * 128] = \
                vw1a[:, c * 128:(c + 1) * 128]
        w2cat.append(np.ascontiguousarray(
            w2.reshape(KC[i], 128, r).transpose(1, 0, 2).reshape(128, -1)
        ).astype(NP_BF16))
        b2blk[:, RBOFF[i]:RBOFF[i] + NRB[i]] = b2.reshape(NRB[i], 128).T
    vw1q = vw1q.astype(NP_BF16)

    ewT1 = np.concatenate([ew.T, np.ones((1, B), np.float32)], axis=0)

    in_maps = []
    for core in range(NCORES):
        er = np.zeros((128, BL), np.float32)
        sl = ewT1[:, core * BL:(core + 1) * BL]
        for t in range(4):
            er[32 * t:32 * t + 17] = sl
        m = {
            "vw1q": vw1q,
            "ewr": er.astype(NP_BF16),
            "b2blk": b2blk,
        }
        for i in range(len(RANKS)):
            m[f"w2_{i}"] = w2cat[i]
        in_maps.append(m)
    return in_maps


def _install_ntff_hook():
    """Provide antenv.axon_hooks if the image lacks it (trace support).

    run_bass_kernel_spmd's axon trace path imports
    antenv.axon_hooks.get_axon_ntff_profile_hook; this container's antenv
    has no such module, so recreate the ctypes-based hook against the
    injected libaxon_pjrt.so (same as trn_agent_boot._ntff_profile_via_ctypes).
    """
    try:
        from antenv.axon_hooks import get_axon_ntff_profile_hook  # noqa: F401
        return
    except ImportError:
        pass
    so_path = "/opt/axon/libaxon_pjrt.so"
    hook = None
    if os.path.exists(so_path):
        lib = ctypes.CDLL(so_path)
        if hasattr(lib, "axon_start_nrt_profile"):
            lib.axon_start_nrt_profile.argtypes = [
                ctypes.POINTER(ctypes.c_int64),
                ctypes.c_size_t,
            ]
            lib.axon_start_nrt_profile.restype = ctypes.c_int64
            lib.axon_stop_nrt_profile.argtypes = [ctypes.c_char_p]
            lib.axon_stop_nrt_profile.restype = ctypes.c_int64

            @contextlib.contextmanager
            def _hook(output_dir, device_ids):
                import jax

                jax.devices()
                if device_ids:
                    ids = (ctypes.c_int64 * len(device_ids))(*device_ids)
                    rc = lib.axon_start_nrt_profile(ids, len(device_ids))
                else:
                    rc = lib.axon_start_nrt_profile(None, 0)
                if rc != 0:
                    raise RuntimeError(f"axon_start_nrt_profile rc={rc}")
                try:
                    yield
                finally:
                    n = lib.axon_stop_nrt_profile(str(output_dir).encode())
                    if n < 0:
                        raise RuntimeError(f"axon_stop_nrt_profile rc={n}")

            hook = _hook

    import antenv

    mod = types.ModuleType("antenv.axon_hooks")
    state = {"hook": hook}
    mod.get_axon_ntff_profile_hook = lambda: state["hook"]
    mod.set_axon_ntff_profile_hook = lambda h: state.__setitem__("hook", h)
    sys.modules["antenv.axon_hooks"] = mod
    antenv.axon_hooks = mod


def run(inputs, trace=False, tmpdir=None):
    """Run the kernel on all 8 cores; returns (full_output, BassKernelResults)."""
    if trace:
        _install_ntff_hook()
    nc = _get_program()
    in_maps = _prepare_inputs(inputs)
    res = run_bass_kernel_spmd(
        nc, in_maps, core_ids=list(range(NCORES)), trace=trace, tmpdir=tmpdir
    )
    # device emits tanh(x)+... transposed [OUT_COLS, BL] in bf16; the *0.1
    # scale and the transpose back to [BL, OUT_COLS] happen here.
    parts = []
    for i in range(NCORES):
        o = res.results[i]["outT"].astype(np.float32)
        parts.append(o.T * np.float32(STRENGTH))
    out = np.ascontiguousarray(np.concatenate(parts, axis=0),
                               dtype=np.float32)
    return out, res


def kernel(**inputs) -> np.ndarray:
    out, _ = run(inputs, trace=False)
    return out


# revision 7
# speedup vs baseline: 1.2132x; 1.0487x over previous
"""Trainium2 Bass kernel for the ExpertVectorSystem MoE-routing problem.

Reference computation (all fp32):
    we = expert_weights @ expert_vectors              # [B, D]
    for each layer i (8 layers, rank r_i):
        h_i   = relu(we @ w1_i + b1_i)                # [B, 2r]
        out_i = tanh(h_i @ w2_i + b2_i) * 0.1         # [B, r]
    out = concat(out_i, axis=-1)                      # [B, sum(r)]

Strategy: data-parallel over the batch across 8 NeuronCores (2048 rows
each); the tiny per-layer MLP weights are replicated.

Key algebra: we = ew @ v has rank <= 16, so h = relu(ew_aug @ vw1_aug)
with vw1_aug = [[v @ w1], [b1]] ([17, 2r], host-folded).  Stage-1
contraction is K=17 instead of 65, so four chunks pack into the PE's
four 32-row tile groups (tile_position row tiling) and stream the same
moving ew columns concurrently: ~4x fewer stage-1 PE cycles.

All matmuls run in bf16 (fp32 PSUM accumulation): same 1-col/cycle PE
rate as fp32r but half the DMA/SBUF traffic, and bf16 stationaries get
Fast Weight Load so LDWEIGHTS hides completely under the matmul stream.
Simulated end-to-end rel err vs the fp32 reference: 4.3e-3 (fp8 would
be 4.9e-2 - fails the 2e-2 gate, so bf16 is the fastest legal dtype).

Stage-2 is computed transposed: out_pre.T[r, batch] accumulated as
(w2 chunk [128, 128-row-block]) stationary x (hT chunk [128, 512])
moving, so every matmul streams N=512 and every LDWEIGHTS (~96ns with
FWL) hides under the 213ns stream.  b2 rides the tanh activation's
per-partition bias port (free); the *0.1 scale and the final
[r, batch] -> [batch, r] transpose happen on the host.

Per-core schedule: per (layer, 512-col batch group) pair, stage-2 runs
r/128 PSUM accumulation groups (kc matmuls each); the next pair's
stage-1 quads are interleaved one-per-accumulation-group so the relu
drains (alternating ScalarE/VectorE) keep pace and PSUM never backs up.
"""

import contextlib
import ctypes
import os
import sys
import types

import numpy as np
import ml_dtypes

import concourse.bass as bass
import concourse.mybir as mybir
import concourse.tile as tile
from concourse.bass_utils import run_bass_kernel_spmd

B = 16384
E = 16
D = 64
RANKS = [256, 384, 512, 640, 768, 896, 1024, 1152]
STRENGTH = 0.1
NCORES = 8
BL = B // NCORES          # 2048 rows per core
GCOLS = 512               # batch columns per group
NGROUPS = BL // GCOLS     # 4

KC = [2 * r // 128 for r in RANKS]        # stage-2 K chunks per layer
NRB = [r // 128 for r in RANKS]           # output 128-row blocks per layer
QC = [(k + 3) // 4 for k in KC]           # stage-1 quads per layer
QOFF = [sum(QC[:i]) for i in range(len(RANKS))]
RBOFF = [sum(NRB[:i]) for i in range(len(RANKS))]
COLOFF = [sum(RANKS[:i]) for i in range(len(RANKS))]
NQ = sum(QC)              # 24 quad columns in vw1q
NRB_TOT = sum(NRB)        # 46

BF16 = mybir.dt.bfloat16
F32 = mybir.dt.float32
NP_BF16 = ml_dtypes.bfloat16

OUT_COLS = sum(RANKS)     # 5888


def _split_excess_waits(nc):
    """Rewrite instructions carrying >1 sync wait.

    The walrus build in this container accepts at most ONE sync wait per
    instruction ("Too many sync wait commands", CoreV*GenImpl
    setupSyncWait), while Tile's wait assignment freely attaches several.
    Hoist the extra waits onto standalone InstEventSemaphore instructions
    (what BassEngine.wait_ge emits) inserted immediately before the
    instruction on the same engine — same-engine program order makes this
    semantically identical.
    """
    n_split = 0
    for f in nc.m.functions:
        for bb in f.blocks:
            out = []
            dirty = False
            for ins in bb.instructions:
                si = ins.sync_info
                waits = list(si.on_wait) if si is not None else []
                if len(waits) > 1:
                    dirty = True
                    for k, w in enumerate(waits[:-1]):
                        out.append(
                            mybir.InstEventSemaphore(
                                name=f"{ins.name}_xw{k}",
                                engine=ins.engine,
                                ins=[],
                                outs=[],
                                sync_info=mybir.SyncInfo(
                                    on_wait=[w], on_update=[]
                                ),
                            )
                        )
                        n_split += 1
                    ins.sync_info = mybir.SyncInfo(
                        on_wait=[waits[-1]], on_update=list(si.on_update)
                    )
                out.append(ins)
            if dirty:
                bb.instructions = out
    return n_split


def _build_program():
    nc = bass.Bass()
    vw1q_d = nc.declare_dram_parameter("vw1q", [128, NQ * 128], BF16,
                                       isOutput=False)
    ewr_d = nc.declare_dram_parameter("ewr", [128, BL], BF16, isOutput=False)
    b2_d = nc.declare_dram_parameter("b2blk", [128, NRB_TOT], F32,
                                     isOutput=False)
    w2_d = [
        nc.declare_dram_parameter(f"w2_{i}", [128, KC[i] * RANKS[i]], BF16,
                                  isOutput=False)
        for i in range(len(RANKS))
    ]
    outT_d = nc.declare_dram_parameter("outT", [OUT_COLS, BL], BF16,
                                       isOutput=True)

    with tile.TileContext(nc) as tc:
        with (
            tc.tile_pool(name="const", bufs=1) as cpool,
            tc.tile_pool(name="hpsum", bufs=6, space="PSUM") as hpsum,
            tc.tile_pool(name="opsum", bufs=2, space="PSUM") as opsum,
            tc.tile_pool(name="w2", bufs=1) as w2pool,
            tc.tile_pool(name="h", bufs=2) as hpool,
            tc.tile_pool(name="osb", bufs=6) as osb,
        ):
            # ---- startup: PE warm-up on a memset tile + sliced DMAs ----
            # Warm-up needs no input data (memset), so it starts at ~0 and
            # runs in the same (32,128) tile mode as the stage-1 quads: the
            # HAM clock gate reaches 8/8 (2.4 GHz) while the first DMAs
            # stream and no mode-switch drain precedes the first real quad.
            wz = cpool.tile([32, 128], BF16, name="warm_zeros")
            nc.vector.memset(wz[:], 0.0)
            for k in range(56):
                warm = opsum.tile([128, 64], F32, tag="op", name=f"warm_{k}")
                nc.tensor.matmul(
                    warm[:], wz[0:17, 0:128], wz[0:17, 0:64],
                    start=True, stop=True, tile_position=(0, 0),
                )

            # first (layer0, group0) slices land first so real work can
            # begin ~2us in; the bulk loads stream behind them.
            vw1q_sb = cpool.tile([128, NQ * 128], BF16, name="vw1q_sb")
            nc.sync.dma_start(vw1q_sb[:, 0:128], vw1q_d[:, 0:128])
            ewr_sb = cpool.tile([128, BL], BF16, name="ewr_sb")
            nc.sync.dma_start(ewr_sb[:, 0:GCOLS], ewr_d[:, 0:GCOLS])
            b2_sb = cpool.tile([128, NRB_TOT], F32, name="b2_sb")

            def load_w2(li):
                r = RANKS[li]
                tiles = []
                for c in range(KC[li]):
                    # even/odd layer tag families: the next layer's DMAs
                    # only wait on the layer-before-last's reads (long
                    # done), so they stream a whole pair ahead instead of
                    # stalling on the current layer's final reads.
                    t = w2pool.tile([128, r], BF16, tag=f"w2_{li % 2}_{c}",
                                    name=f"w2_{li}_{c}")
                    nc.sync.dma_start(t[:], w2_d[li][:, c * r:(c + 1) * r])
                    tiles.append(t)
                return tiles

            w2_sb = {0: load_w2(0)}
            nc.sync.dma_start(b2_sb[:], b2_d[:])
            nc.sync.dma_start(vw1q_sb[:, 128:NQ * 128],
                              vw1q_d[:, 128:NQ * 128])
            nc.sync.dma_start(ewr_sb[:, GCOLS:BL], ewr_d[:, GCOLS:BL])

            # ---- stage 1: h chunks via 4-packed 32-row-tile matmuls ----
            def stage1_quads(li, g, h_sb):
                """Yield thunks; each emits one quad of K=17 matmuls into
                the PE's four 32-row tile groups (concurrent on HW) plus
                their relu drains split across ScalarE/VectorE."""
                qo = QOFF[li]
                for q in range(QC[li]):
                    def unit(q=q):
                        nt = min(4, KC[li] - 4 * q)
                        hps = []
                        for t in range(nt):
                            hp = hpsum.tile([128, GCOLS], F32, tag="hp",
                                            name=f"hp_{li}_{g}_{4*q+t}")
                            nc.tensor.matmul(
                                hp[:],
                                vw1q_sb[32 * t:32 * t + 17,
                                        (qo + q) * 128:(qo + q + 1) * 128],
                                ewr_sb[32 * t:32 * t + 17,
                                       g * GCOLS:(g + 1) * GCOLS],
                                start=True, stop=True,
                                tile_position=(32 * t, 0),
                            )
                            hps.append(hp)
                        for t, hp in enumerate(hps):
                            c = 4 * q + t
                            ht = hpool.tile([128, GCOLS], BF16, tag=f"h_{c}",
                                            name=f"h_{li}_{g}_{c}")
                            # split each relu drain across both engines so
                            # the hp PSUM bank recycles in ~340ns and the
                            # next quad never stalls on bank availability
                            half = GCOLS // 2
                            nc.scalar.activation(
                                ht[:, 0:half], hp[:, 0:half],
                                mybir.ActivationFunctionType.Relu,
                            )
                            nc.vector.tensor_scalar_max(
                                ht[:, half:GCOLS], hp[:, half:GCOLS], 0.0
                            )
                            h_sb.append(ht)
                    yield unit

            # ---- main sweep over (layer, batch-group) pairs ----
            pairs = [(li, g) for li in range(len(RANKS))
                     for g in range(NGROUPS)]
            h_cur = []
            for u in stage1_quads(0, 0, h_cur):
                u()
            for idx, (li, g) in enumerate(pairs):
                r = RANKS[li]
                kc = KC[li]
                nrb = NRB[li]
                nxt = pairs[idx + 1] if idx + 1 < len(pairs) else None
                h_nxt = []
                units = []
                if nxt is not None:
                    nli, ng = nxt
                    if nli != li:
                        w2_sb[nli] = load_w2(nli)
                    units = list(stage1_quads(nli, ng, h_nxt))
                ui = 0
                for rb in range(nrb):
                    op = opsum.tile([128, GCOLS], F32, tag="op",
                                    name=f"op_{li}_{g}_{rb}")
                    for c in range(kc):
                        nc.tensor.matmul(
                            op[:],
                            w2_sb[li][c][:, rb * 128:(rb + 1) * 128],
                            h_cur[c][:],
                            start=(c == 0), stop=(c == kc - 1),
                        )
                    # next pair's stage-1 quad between accumulation groups:
                    # its 4 relu drains get a full group (>= kc*213ns) to
                    # clear the 4 hp banks before the next quad needs them.
                    if ui < len(units):
                        units[ui]()
                        ui += 1
                    ot = osb.tile([128, GCOLS], BF16, tag="ot",
                                  name=f"ot_{li}_{g}_{rb}")
                    rbg = RBOFF[li] + rb
                    nc.scalar.activation(
                        ot[:], op[:], mybir.ActivationFunctionType.Tanh,
                        bias=b2_sb[:, rbg:rbg + 1],
                    )
                    row0 = COLOFF[li] + rb * 128
                    nc.sync.dma_start(
                        outT_d[row0:row0 + 128, g * GCOLS:(g + 1) * GCOLS],
                        ot[:],
                    )
                for u in units[ui:]:
                    u()
                h_cur = h_nxt
    _split_excess_waits(nc)
    return nc


_CACHE = {}


def _get_program():
    if "p" not in _CACHE:
        _CACHE["p"] = _build_program()
    return _CACHE["p"]


def _prepare_inputs(inputs):
    """Host-side marshalling: fold v@w1+b1 into the quad-packed stage-1
    stationary, chunk w2, build the replicated [ew^T; ones] bands."""
    ew = np.asarray(inputs["expert_weights"], dtype=np.float32)
    v = np.asarray(inputs["expert_vectors"], dtype=np.float32)

    vw1q = np.zeros((128, NQ * 128), np.float32)
    b2blk = np.zeros((128, NRB_TOT), np.float32)
    w2cat = []
    for i, r in enumerate(RANKS):
        w1 = np.asarray(inputs[f"w1_{i}"], dtype=np.float32)   # [D, 2r]
        b1 = np.asarray(inputs[f"b1_{i}"], dtype=np.float32)   # [2r]
        w2 = np.asarray(inputs[f"w2_{i}"], dtype=np.float32)   # [2r, r]
        b2 = np.asarray(inputs[f"b2_{i}"], dtype=np.float32)   # [r]
        vw1a = np.concatenate([v @ w1, b1[None, :]], axis=0)   # [17, 2r]
        for c in range(KC[i]):
            q, t = divmod(c, 4)
            vw1q[32 * t:32 * t + 17,
                 (QOFF[i] + q) * 128:(QOFF[i] + q + 1) * 128] = \
                vw1a[:, c * 128:(c + 1) * 128]
        w2cat.append(np.ascontiguousarray(
            w2.reshape(KC[i], 128, r).transpose(1, 0, 2).reshape(128, -1)
        ).astype(NP_BF16))
        b2blk[:, RBOFF[i]:RBOFF[i] + NRB[i]] = b2.reshape(NRB[i], 128).T
    vw1q = vw1q.astype(NP_BF16)

    ewT1 = np.concatenate([ew.T, np.ones((1, B), np.float32)], axis=0)

    in_maps = []
    for core in range(NCORES):
        er = np.zeros((128, BL), np.float32)
        sl = ewT1[:, core * BL:(core + 1) * BL]
        for t in range(4):
            er[32 * t:32 * t + 17] = sl
        m = {
            "vw1q": vw1q,
            "ewr": er.astype(NP_BF16),
            "b2blk": b2blk,
        }
        for i in range(len(RANKS)):
            m[f"w2_{i}"] = w2cat[i]
        in_maps.append(m)
    return in_maps


def _install_ntff_hook():
    """Provide antenv.axon_hooks if the image lacks it (trace support).

    run_bass_kernel_spmd's axon trace path imports
    antenv.axon_hooks.get_axon_ntff_profile_hook; this container's antenv
    has no such module, so recreate the ctypes-based hook against the
    injected libaxon_pjrt.so (same as trn_agent_boot._ntff_profile_via_ctypes).
    """
    try:
        from antenv.axon_hooks import get_axon_ntff_profile_hook  # noqa: F401
        return
    except ImportError:
        pass
    so_path = "/opt/axon/libaxon_pjrt.so"
    hook = None
    if os.path.exists(so_path):
        lib = ctypes.CDLL(so_path)
        if hasattr(lib, "axon_start_nrt_profile"):
            lib.axon_start_nrt_profile.argtypes = [
                ctypes.POINTER(ctypes.c_int64),
                ctypes.c_size_t,
            ]
            lib.axon_start_nrt_profile.restype = ctypes.c_int64
            lib.axon_stop_nrt_profile.argtypes = [ctypes.c_char_p]
            lib.axon_stop_nrt_profile.restype = ctypes.c_int64

            @contextlib.contextmanager
            def _hook(output_dir, device_ids):
                import jax

                jax.devices()
                if device_ids:
                    ids = (ctypes.c_int64 * len(device_ids))(*device_ids)
                    rc = lib.axon_start_nrt_profile(ids, len(device_ids))
                else:
                    rc = lib.axon_start_nrt_profile(None, 0)
                if rc != 0:
                    raise RuntimeError(f"axon_start_nrt_profile rc={rc}")
                try:
                    yield
                finally:
                    n = lib.axon_stop_nrt_profile(str(output_dir).encode())
                    if n < 0:
                        raise RuntimeError(f"axon_stop_nrt_profile rc={n}")

            hook = _hook

    import antenv

    mod = types.ModuleType("antenv.axon_hooks")
    state = {"hook": hook}
    mod.get_axon_ntff_profile_hook = lambda: state["hook"]
    mod.set_axon_ntff_profile_hook = lambda h: state.__setitem__("hook", h)
    sys.modules["antenv.axon_hooks"] = mod
    antenv.axon_hooks = mod


def run(inputs, trace=False, tmpdir=None):
    """Run the kernel on all 8 cores; returns (full_output, BassKernelResults)."""
    if trace:
        _install_ntff_hook()
    nc = _get_program()
    in_maps = _prepare_inputs(inputs)
    res = run_bass_kernel_spmd(
        nc, in_maps, core_ids=list(range(NCORES)), trace=trace, tmpdir=tmpdir
    )
    # device emits tanh(x)+... transposed [OUT_COLS, BL] in bf16; the *0.1
    # scale and the transpose back to [BL, OUT_COLS] happen here.
    parts = []
    for i in range(NCORES):
        o = res.results[i]["outT"].astype(np.float32)
        parts.append(o.T * np.float32(STRENGTH))
    out = np.ascontiguousarray(np.concatenate(parts, axis=0),
                               dtype=np.float32)
    return out, res


def kernel(**inputs) -> np.ndarray:
    out, _ = run(inputs, trace=False)
    return out


# revision 13
# speedup vs baseline: 1.2157x; 1.0020x over previous
"""Trainium2 Bass kernel for the ExpertVectorSystem MoE-routing problem.

Reference computation (all fp32):
    we = expert_weights @ expert_vectors              # [B, D]
    for each layer i (8 layers, rank r_i):
        h_i   = relu(we @ w1_i + b1_i)                # [B, 2r]
        out_i = tanh(h_i @ w2_i + b2_i) * 0.1         # [B, r]
    out = concat(out_i, axis=-1)                      # [B, sum(r)]

Strategy: data-parallel over the batch across 8 NeuronCores (2048 rows
each); the tiny per-layer MLP weights are replicated.

Key algebra: we = ew @ v has rank <= 16, so h = relu(ew_aug @ vw1_aug)
with vw1_aug = [[v @ w1], [b1]] ([17, 2r], host-folded).  Stage-1
contraction is K=17 instead of 65, so four chunks pack into the PE's
four 32-row tile groups (tile_position row tiling) and stream the same
moving ew columns concurrently: ~4x fewer stage-1 PE cycles.

All matmuls run in bf16 (fp32 PSUM accumulation): same 1-col/cycle PE
rate as fp32r but half the DMA/SBUF traffic, and bf16 stationaries get
Fast Weight Load so LDWEIGHTS hides completely under the matmul stream.
Simulated end-to-end rel err vs the fp32 reference: 4.3e-3 (fp8 would
be 4.9e-2 - fails the 2e-2 gate, so bf16 is the fastest legal dtype).

Stage-2 is computed transposed: out_pre.T[r, batch] accumulated as
(w2 chunk [128, 128-row-block]) stationary x (hT chunk [128, 512])
moving, so every matmul streams N=512 and every LDWEIGHTS (~96ns with
FWL) hides under the 213ns stream.  b2 rides the tanh activation's
per-partition bias port (free); the *0.1 scale and the final
[r, batch] -> [batch, r] transpose happen on the host.

Per-core schedule: per (layer, 512-col batch group) pair, stage-2 runs
r/128 PSUM accumulation groups (kc matmuls each); the next pair's
stage-1 quads are interleaved one-per-accumulation-group so the relu
drains (alternating ScalarE/VectorE) keep pace and PSUM never backs up.
"""

import contextlib
import ctypes
import os
import sys
import types

import numpy as np
import ml_dtypes

import concourse.bass as bass
import concourse.mybir as mybir
import concourse.tile as tile
from concourse.bass_utils import run_bass_kernel_spmd

B = 16384
E = 16
D = 64
RANKS = [256, 384, 512, 640, 768, 896, 1024, 1152]
STRENGTH = 0.1
NCORES = 8
BL = B // NCORES          # 2048 rows per core
GCOLS = 512               # batch columns per group
NGROUPS = BL // GCOLS     # 4

KC = [2 * r // 128 for r in RANKS]        # stage-2 K chunks per layer
NRB = [r // 128 for r in RANKS]           # output 128-row blocks per layer
QC = [(k + 3) // 4 for k in KC]           # stage-1 quads per layer
QOFF = [sum(QC[:i]) for i in range(len(RANKS))]
RBOFF = [sum(NRB[:i]) for i in range(len(RANKS))]
COLOFF = [sum(RANKS[:i]) for i in range(len(RANKS))]
NQ = sum(QC)              # 24 quad columns in vw1q
NRB_TOT = sum(NRB)        # 46

BF16 = mybir.dt.bfloat16
F32 = mybir.dt.float32
NP_BF16 = ml_dtypes.bfloat16

OUT_COLS = sum(RANKS)     # 5888


def _split_excess_waits(nc):
    """Rewrite instructions carrying >1 sync wait.

    The walrus build in this container accepts at most ONE sync wait per
    instruction ("Too many sync wait commands", CoreV*GenImpl
    setupSyncWait), while Tile's wait assignment freely attaches several.
    Hoist the extra waits onto standalone InstEventSemaphore instructions
    (what BassEngine.wait_ge emits) inserted immediately before the
    instruction on the same engine — same-engine program order makes this
    semantically identical.
    """
    n_split = 0
    for f in nc.m.functions:
        for bb in f.blocks:
            out = []
            dirty = False
            for ins in bb.instructions:
                si = ins.sync_info
                waits = list(si.on_wait) if si is not None else []
                if len(waits) > 1:
                    dirty = True
                    for k, w in enumerate(waits[:-1]):
                        out.append(
                            mybir.InstEventSemaphore(
                                name=f"{ins.name}_xw{k}",
                                engine=ins.engine,
                                ins=[],
                                outs=[],
                                sync_info=mybir.SyncInfo(
                                    on_wait=[w], on_update=[]
                                ),
                            )
                        )
                        n_split += 1
                    ins.sync_info = mybir.SyncInfo(
                        on_wait=[waits[-1]], on_update=list(si.on_update)
                    )
                out.append(ins)
            if dirty:
                bb.instructions = out
    return n_split


def _build_program():
    nc = bass.Bass()
    vw1q_d = nc.declare_dram_parameter("vw1q", [128, NQ * 128], BF16,
                                       isOutput=False)
    ewr_d = nc.declare_dram_parameter("ewr", [128, BL], BF16, isOutput=False)
    b2_d = nc.declare_dram_parameter("b2blk", [128, NRB_TOT], F32,
                                     isOutput=False)
    w2_d = [
        nc.declare_dram_parameter(f"w2_{i}", [128, KC[i] * RANKS[i]], BF16,
                                  isOutput=False)
        for i in range(len(RANKS))
    ]
    outT_d = nc.declare_dram_parameter("outT", [OUT_COLS, BL], BF16,
                                       isOutput=True)

    with tile.TileContext(nc) as tc:
        with (
            tc.tile_pool(name="const", bufs=1) as cpool,
            tc.tile_pool(name="hpsum", bufs=6, space="PSUM") as hpsum,
            tc.tile_pool(name="opsum", bufs=2, space="PSUM") as opsum,
            tc.tile_pool(name="w2", bufs=1) as w2pool,
            tc.tile_pool(name="h", bufs=2) as hpool,
            tc.tile_pool(name="osb", bufs=6) as osb,
        ):
            # ---- startup: PE warm-up on a memset tile + sliced DMAs ----
            # Warm-up needs no input data (memset), so it starts at ~0 and
            # runs in the same (32,128) tile mode as the stage-1 quads: the
            # HAM clock gate reaches 8/8 (2.4 GHz) while the first DMAs
            # stream and no mode-switch drain precedes the first real quad.
            # a ~6us framework preamble (engine barrier + const loads) runs
            # before any user instruction, so only a short warm bridge is
            # needed until the first input slices land (~1.5us later).
            wz = cpool.tile([32, 128], BF16, name="warm_zeros")
            nc.vector.memset(wz[:], 0.0)
            for k in range(12):
                warm = opsum.tile([128, 64], F32, tag="op", name=f"warm_{k}")
                nc.tensor.matmul(
                    warm[:], wz[0:17, 0:128], wz[0:17, 0:64],
                    start=True, stop=True, tile_position=(0, 0),
                )

            # first (layer0, group0) slices land first so real work can
            # begin ~2us in; the bulk loads stream behind them.
            vw1q_sb = cpool.tile([128, NQ * 128], BF16, name="vw1q_sb")
            nc.sync.dma_start(vw1q_sb[:, 0:128], vw1q_d[:, 0:128])
            ewr_sb = cpool.tile([128, BL], BF16, name="ewr_sb")
            nc.sync.dma_start(ewr_sb[:, 0:GCOLS], ewr_d[:, 0:GCOLS])
            b2_sb = cpool.tile([128, NRB_TOT], F32, name="b2_sb")

            def load_w2(li):
                r = RANKS[li]
                tiles = []
                for c in range(KC[li]):
                    # even/odd layer tag families: the next layer's DMAs
                    # only wait on the layer-before-last's reads (long
                    # done), so they stream a whole pair ahead instead of
                    # stalling on the current layer's final reads.
                    t = w2pool.tile([128, r], BF16, tag=f"w2_{li % 2}_{c}",
                                    name=f"w2_{li}_{c}")
                    nc.sync.dma_start(t[:], w2_d[li][:, c * r:(c + 1) * r])
                    tiles.append(t)
                return tiles

            w2_sb = {0: load_w2(0)}
            nc.sync.dma_start(b2_sb[:], b2_d[:])
            nc.sync.dma_start(vw1q_sb[:, 128:NQ * 128],
                              vw1q_d[:, 128:NQ * 128])
            nc.sync.dma_start(ewr_sb[:, GCOLS:BL], ewr_d[:, GCOLS:BL])

            # ---- stage 1: h chunks via 4-packed 32-row-tile matmuls ----
            def stage1_quads(li, g, h_sb, act_cols=256):
                """Yield thunks; each emits one quad of K=17 matmuls into
                the PE's four 32-row tile groups (concurrent on HW) plus
                their relu drains split across ScalarE/VectorE.  act_cols
                sets ScalarE's share of each drain (it also runs the tanh,
                so small-kc host pairs give it a lighter slice)."""
                qo = QOFF[li]
                for q in range(QC[li]):
                    def unit(q=q):
                        nt = min(4, KC[li] - 4 * q)
                        hps = []
                        for t in range(nt):
                            hp = hpsum.tile([128, GCOLS], F32, tag="hp",
                                            name=f"hp_{li}_{g}_{4*q+t}")
                            nc.tensor.matmul(
                                hp[:],
                                vw1q_sb[32 * t:32 * t + 17,
                                        (qo + q) * 128:(qo + q + 1) * 128],
                                ewr_sb[32 * t:32 * t + 17,
                                       g * GCOLS:(g + 1) * GCOLS],
                                start=True, stop=True,
                                tile_position=(32 * t, 0),
                            )
                            hps.append(hp)
                        for t, hp in enumerate(hps):
                            c = 4 * q + t
                            ht = hpool.tile([128, GCOLS], BF16, tag=f"h_{c}",
                                            name=f"h_{li}_{g}_{c}")
                            # split each relu drain across both engines so
                            # the hp PSUM bank recycles fast and the next
                            # quad never stalls on bank availability
                            nc.scalar.activation(
                                ht[:, 0:act_cols], hp[:, 0:act_cols],
                                mybir.ActivationFunctionType.Relu,
                            )
                            nc.vector.tensor_scalar_max(
                                ht[:, act_cols:GCOLS], hp[:, act_cols:GCOLS],
                                0.0,
                            )
                            h_sb.append(ht)
                    yield unit

            # ---- main sweep over (layer, batch-group) pairs ----
            pairs = [(li, g) for li in range(len(RANKS))
                     for g in range(NGROUPS)]
            h_cur = []
            for u in stage1_quads(0, 0, h_cur, act_cols=128):
                u()
            for idx, (li, g) in enumerate(pairs):
                r = RANKS[li]
                kc = KC[li]
                nrb = NRB[li]
                nxt = pairs[idx + 1] if idx + 1 < len(pairs) else None
                h_nxt = []
                units = []
                if nxt is not None:
                    nli, ng = nxt
                    if nli != li:
                        w2_sb[nli] = load_w2(nli)
                    units = list(stage1_quads(
                        nli, ng, h_nxt,
                        act_cols=(128 if kc <= 8 else 256),
                    ))
                ui = 0
                for rb in range(nrb):
                    op = opsum.tile([128, GCOLS], F32, tag="op",
                                    name=f"op_{li}_{g}_{rb}")
                    for c in range(kc):
                        nc.tensor.matmul(
                            op[:],
                            w2_sb[li][c][:, rb * 128:(rb + 1) * 128],
                            h_cur[c][:],
                            start=(c == 0), stop=(c == kc - 1),
                        )
                    # next pair's stage-1 quad between accumulation groups:
                    # its 4 relu drains get a full group (>= kc*213ns) to
                    # clear the 4 hp banks before the next quad needs them.
                    if ui < len(units):
                        units[ui]()
                        ui += 1
                    ot = osb.tile([128, GCOLS], BF16, tag="ot",
                                  name=f"ot_{li}_{g}_{rb}")
                    rbg = RBOFF[li] + rb
                    nc.scalar.activation(
                        ot[:], op[:], mybir.ActivationFunctionType.Tanh,
                        bias=b2_sb[:, rbg:rbg + 1],
                    )
                    row0 = COLOFF[li] + rb * 128
                    nc.sync.dma_start(
                        outT_d[row0:row0 + 128, g * GCOLS:(g + 1) * GCOLS],
                        ot[:],
                    )
                for u in units[ui:]:
                    u()
                h_cur = h_nxt
    _split_excess_waits(nc)
    return nc


_CACHE = {}


def _get_program():
    if "p" not in _CACHE:
        _CACHE["p"] = _build_program()
    return _CACHE["p"]


def _prepare_inputs(inputs):
    """Host-side marshalling: fold v@w1+b1 into the quad-packed stage-1
    stationary, chunk w2, build the replicated [ew^T; ones] bands."""
    ew = np.asarray(inputs["expert_weights"], dtype=np.float32)
    v = np.asarray(inputs["expert_vectors"], dtype=np.float32)

    vw1q = np.zeros((128, NQ * 128), np.float32)
    b2blk = np.zeros((128, NRB_TOT), np.float32)
    w2cat = []
    for i, r in enumerate(RANKS):
        w1 = np.asarray(inputs[f"w1_{i}"], dtype=np.float32)   # [D, 2r]
        b1 = np.asarray(inputs[f"b1_{i}"], dtype=np.float32)   # [2r]
        w2 = np.asarray(inputs[f"w2_{i}"], dtype=np.float32)   # [2r, r]
        b2 = np.asarray(inputs[f"b2_{i}"], dtype=np.float32)   # [r]
        vw1a = np.concatenate([v @ w1, b1[None, :]], axis=0)   # [17, 2r]
        for c in range(KC[i]):
            q, t = divmod(c, 4)
            vw1q[32 * t:32 * t + 17,
                 (QOFF[i] + q) * 128:(QOFF[i] + q + 1) * 128] = \
                vw1a[:, c * 128:(c + 1) * 128]
        w2cat.append(np.ascontiguousarray(
            w2.reshape(KC[i], 128, r).transpose(1, 0, 2).reshape(128, -1)
        ).astype(NP_BF16))
        b2blk[:, RBOFF[i]:RBOFF[i] + NRB[i]] = b2.reshape(NRB[i], 128).T
    vw1q = vw1q.astype(NP_BF16)

    ewT1 = np.concatenate([ew.T, np.ones((1, B), np.float32)], axis=0)

    in_maps = []
    for core in range(NCORES):
        er = np.zeros((128, BL), np.float32)
        sl = ewT1[:, core * BL:(core + 1) * BL]
        for t in range(4):
            er[32 * t:32 * t + 17] = sl
        m = {
            "vw1q": vw1q,
            "ewr": er.astype(NP_BF16),
            "b2blk": b2blk,
        }
        for i in range(len(RANKS)):
            m[f"w2_{i}"] = w2cat[i]
        in_maps.append(m)
    return in_maps


def _install_ntff_hook():
    """Provide antenv.axon_hooks if the image lacks it (trace support).

    run_bass_kernel_spmd's axon trace path imports
    antenv.axon_hooks.get_axon_ntff_profile_hook; this container's antenv
    has no such module, so recreate the ctypes-based hook against the
    injected libaxon_pjrt.so (same as trn_agent_boot._ntff_profile_via_ctypes).
    """
    try:
        from antenv.axon_hooks import get_axon_ntff_profile_hook  # noqa: F401
        return
    except ImportError:
        pass
    so_path = "/opt/axon/libaxon_pjrt.so"
    hook = None
    if os.path.exists(so_path):
        lib = ctypes.CDLL(so_path)
        if hasattr(lib, "axon_start_nrt_profile"):
            lib.axon_start_nrt_profile.argtypes = [
                ctypes.POINTER(ctypes.c_int64),
                ctypes.c_size_t,
            ]
            lib.axon_start_nrt_profile.restype = ctypes.c_int64
            lib.axon_stop_nrt_profile.argtypes = [ctypes.c_char_p]
            lib.axon_stop_nrt_profile.restype = ctypes.c_int64

            @contextlib.contextmanager
            def _hook(output_dir, device_ids):
                import jax

                jax.devices()
                if device_ids:
                    ids = (ctypes.c_int64 * len(device_ids))(*device_ids)
                    rc = lib.axon_start_nrt_profile(ids, len(device_ids))
                else:
                    rc = lib.axon_start_nrt_profile(None, 0)
                if rc != 0:
                    raise RuntimeError(f"axon_start_nrt_profile rc={rc}")
                try:
                    yield
                finally:
                    n = lib.axon_stop_nrt_profile(str(output_dir).encode())
                    if n < 0:
                        raise RuntimeError(f"axon_stop_nrt_profile rc={n}")

            hook = _hook

    import antenv

    mod = types.ModuleType("antenv.axon_hooks")
    state = {"hook": hook}
    mod.get_axon_ntff_profile_hook = lambda: state["hook"]
    mod.set_axon_ntff_profile_hook = lambda h: state.__setitem__("hook", h)
    sys.modules["antenv.axon_hooks"] = mod
    antenv.axon_hooks = mod


def run(inputs, trace=False, tmpdir=None):
    """Run the kernel on all 8 cores; returns (full_output, BassKernelResults)."""
    if trace:
        _install_ntff_hook()
    nc = _get_program()
    in_maps = _prepare_inputs(inputs)
    res = run_bass_kernel_spmd(
        nc, in_maps, core_ids=list(range(NCORES)), trace=trace, tmpdir=tmpdir
    )
    # device emits tanh(x)+... transposed [OUT_COLS, BL] in bf16; the *0.1
    # scale and the transpose back to [BL, OUT_COLS] happen here.
    parts = []
    for i in range(NCORES):
        o = res.results[i]["outT"].astype(np.float32)
        parts.append(o.T * np.float32(STRENGTH))
    out = np.ascontiguousarray(np.concatenate(parts, axis=0),
                               dtype=np.float32)
    return out, res


def kernel(**inputs) -> np.ndarray:
    out, _ = run(inputs, trace=False)
    return out
